# revision 12
# baseline (speedup 1.0000x reference)
"""Trainium2 Bass kernel for nn_HeatmapLayer: separable Gaussian heatmaps.

Reference math (per batch b, class c):
    mx = labels[b, 2c] * H ; my = labels[b, 2c+1] * W          (H = W = 384)
    sigma = H * exp(log_weight)
    dx2[h] = (h - mx)^2 / sigma        ; normalized by its min over h
    dy2[w] = (w - my)^2 / (20 * sigma) ; normalized by its min over w
    out[b,c,h,w] = exp(-0.5*(dx2[h] + dy2[w])) = ex[h] * ey[w]

Each (b,c) heatmap is a rank-1 outer product of two 384-length
profiles.  Per core (pure data parallel over batch: 2 batches = 12
(b,c) pairs per core).  The log-domain x-profile lxm (with both
min-normalization corrections folded in) is computed on a [12, 384]
tile (partition = pair) and PE-transposed into per-partition scalars;
both it and its exponential exm are transposed: LXT/EXT [128, 3, 12].

Two per-pair paths, chosen to balance the Vector and Scalar engines
(every output element must be written once by some engine; the split
puts ~16us on each):

  * DVE path (8 pairs):  ey_p(w) = U(w) * exp(a_p*w + c_p) with
    U(w) = exp(sc_y*w^2) shared across pairs; per pair one ACT Exp
    (linear arg via per-partition scale/bias), one DVE tensor_tensor
    (U*E_p), then 3 DVE tensor_scalar multiplies by EXT.
  * ACT path (4 pairs):  one ACT Square -> sq_y, then per chunk one
    ACT Exp(sq_y*sc_y + LXT[:,c,p]) writes the final chunk directly.

Exp args stay within +-54, far from f32 limits, because
min (w-my)^2 <= 1 and sc_y*384^2 <= 54 for Xavier-bounded log_weight.

Output staged in SBUF, one ~576KB HWDGE DMA per pair (the ~20us
per-core HBM roofline).  x is only used for its shape; it is never
transferred to the device.
"""

import numpy as np
from contextlib import ExitStack

import concourse.bacc as bacc
import concourse.bass as bass
import concourse.tile as tile
from concourse import mybir
from concourse.bass_utils import run_bass_kernel_spmd
from concourse.masks import make_identity

B, CH, H, W = 16, 3, 384, 384
NCLS = 6
N_CORES = 8
BPC = B // N_CORES            # batches per core = 2
PAIRS = BPC * NCLS            # (b,c) pairs per core = 12
P = 128
CHUNKS = H // P               # 3
LN_H = float(np.log(H))
F32 = mybir.dt.float32

N_ACT_PAIRS = 4               # pairs on the all-ACT path (interleaved)
ACT_PAIRS = set(range(2, 2 + 3 * N_ACT_PAIRS, 3))  # {2, 5, 8, 11}


def build_bass() -> bass.Bass:
    nc = bacc.Bacc("TRN2", target_bir_lowering=False, debug=False,
                   num_devices=N_CORES)
    labels = nc.dram_tensor("labels", [BPC, 2 * NCLS], F32,
                            kind="ExternalInput")
    logw = nc.dram_tensor("log_weight", [1, 1], F32, kind="ExternalInput")
    out = nc.dram_tensor("out", [PAIRS * H, W], F32, kind="ExternalOutput")

    with ExitStack() as ctx:
        tc = ctx.enter_context(tile.TileContext(nc))
        singles = ctx.enter_context(tc.tile_pool(name="singles", bufs=1))
        psum = ctx.enter_context(tc.tile_pool(name="psum", bufs=3,
                                              space="PSUM"))
        ybuf = ctx.enter_context(tc.tile_pool(name="ybuf", bufs=6))
        stage = ctx.enter_context(tc.tile_pool(name="stage", bufs=8))

        # ---- shared grid: iota in f32 (0..383 exact) ---------------------
        iog = singles.tile([P, W], F32)
        nc.gpsimd.iota(iog, pattern=[[1, W]], base=0, channel_multiplier=0,
                       allow_small_or_imprecise_dtypes=True)

        # ---- small-tile setup: pairs on partitions 0..11 -----------------
        lab = singles.tile([PAIRS, 2], F32)
        nc.sync.dma_start(
            out=lab,
            in_=labels[:, :].rearrange("b (q two) -> (b q) two", two=2),
        )
        lwb = singles.tile([PAIRS, 1], F32)
        nc.gpsimd.dma_start(out=lwb, in_=logw[:, :].to_broadcast((PAIRS, 1)))

        # neg_m[:,0] = -mx, neg_m[:,1] = -my
        neg_m = singles.tile([PAIRS, 2], F32)
        nc.vector.tensor_scalar_mul(out=neg_m, in0=lab, scalar1=-float(H))

        # inv_s = 1/sigma = exp(-log_weight - ln(H))
        nlw = singles.tile([PAIRS, 1], F32)
        nc.vector.tensor_scalar(out=nlw, in0=lwb, scalar1=-1.0,
                                scalar2=-LN_H, op0=mybir.AluOpType.mult,
                                op1=mybir.AluOpType.add)
        inv_s = singles.tile([PAIRS, 1], F32)
        nc.scalar.activation(out=inv_s, in_=nlw,
                             func=mybir.ActivationFunctionType.Exp,
                             bias=0.0, scale=1.0)
        # sc columns: 0: -inv_s/2 (x exp scale), 1: +inv_s/2, 2: +inv_s/40
        sc = singles.tile([PAIRS, 3], F32)
        for i, m in enumerate((-0.5, 0.5, 0.025)):
            nc.vector.tensor_scalar_mul(out=sc[:, i:i + 1], in0=inv_s,
                                        scalar1=m)

        sqx = singles.tile([PAIRS, W], F32)
        sqy12 = singles.tile([PAIRS, W], F32)
        nc.scalar.activation(out=sqx, in_=iog[:PAIRS, :],
                             func=mybir.ActivationFunctionType.Square,
                             bias=neg_m[:, 0:1], scale=1.0)
        nc.scalar.activation(out=sqy12, in_=iog[:PAIRS, :],
                             func=mybir.ActivationFunctionType.Square,
                             bias=neg_m[:, 1:2], scale=1.0)
        mnx = singles.tile([PAIRS, 1], F32)
        mny = singles.tile([PAIRS, 1], F32)
        nc.vector.tensor_reduce(out=mnx, in_=sqx, axis=mybir.AxisListType.X,
                                op=mybir.AluOpType.min)
        nc.vector.tensor_reduce(out=mny, in_=sqy12, axis=mybir.AxisListType.X,
                                op=mybir.AluOpType.min)
        # fold BOTH min corrections into the x profile (log domain):
        #   lxm[h] = sc_x*sqx[h] + inv_s/2*mnx + inv_s/40*mny
        bx = singles.tile([PAIRS, 1], F32)
        by = singles.tile([PAIRS, 1], F32)
        b2 = singles.tile([PAIRS, 1], F32)
        nc.vector.tensor_mul(out=bx, in0=mnx, in1=sc[:, 1:2])
        nc.vector.tensor_mul(out=by, in0=mny, in1=sc[:, 2:3])
        nc.vector.tensor_add(out=b2, in0=bx, in1=by)
        lxm = singles.tile([PAIRS, W], F32)
        nc.vector.tensor_scalar(out=lxm, in0=sqx, scalar1=sc[:, 0:1],
                                scalar2=b2, op0=mybir.AluOpType.mult,
                                op1=mybir.AluOpType.add)
        exm = singles.tile([PAIRS, W], F32)
        nc.scalar.activation(out=exm, in_=lxm,
                             func=mybir.ActivationFunctionType.Exp,
                             bias=0.0, scale=1.0)

        # ---- PE-transpose both x profiles to per-partition scalars -------
        ident = singles.tile([PAIRS, PAIRS], F32)
        make_identity(nc, ident)
        ext = singles.tile([P, CHUNKS, PAIRS], F32)
        lxt = singles.tile([P, CHUNKS, PAIRS], F32)
        for c in range(CHUNKS):
            for src, dst in ((exm, ext), (lxm, lxt)):
                pt = psum.tile([P, PAIRS], F32)
                nc.tensor.transpose(pt, src[:, c * P:(c + 1) * P], ident)
                nc.vector.tensor_copy(out=dst[:, c, :], in_=pt)

        # ---- y-side coefficients on all 128 partitions -------------------
        lab128 = singles.tile([P, BPC * 2 * NCLS], F32)
        lsrc = labels[:, :].rearrange("b t -> (b t)")
        nc.gpsimd.dma_start(
            out=lab128,
            in_=bass.AP(tensor=lsrc.tensor, offset=lsrc.offset,
                        ap=[[0, P], [1, BPC * 2 * NCLS]]),
        )
        lw128 = singles.tile([P, 1], F32)
        nc.gpsimd.dma_start(out=lw128, in_=logw[:, :].to_broadcast((P, 1)))

        # nmy128[:, p] = -my_p on every partition
        nmy128 = singles.tile([P, PAIRS], F32)
        nc.vector.tensor_scalar_mul(
            out=nmy128,
            in0=lab128[:, :].rearrange("p (q two) -> p q two", two=2)[:, :, 1],
            scalar1=-float(H))
        t128 = singles.tile([P, 1], F32)
        nc.vector.tensor_scalar(out=t128, in0=lw128, scalar1=-1.0,
                                scalar2=-LN_H, op0=mybir.AluOpType.mult,
                                op1=mybir.AluOpType.add)
        inv128 = singles.tile([P, 1], F32)
        nc.scalar.activation(out=inv128, in_=t128,
                             func=mybir.ActivationFunctionType.Exp,
                             bias=0.0, scale=1.0)
        scy128 = singles.tile([P, 1], F32)       # sc_y = -inv_s/40
        nc.vector.tensor_scalar_mul(out=scy128, in0=inv128, scalar1=-0.025)
        scy2 = singles.tile([P, 1], F32)         # 2*sc_y
        nc.vector.tensor_scalar_mul(out=scy2, in0=scy128, scalar1=2.0)

        # a_p = -2*sc_y*my_p = 2*sc_y*(-my_p);  c_p = sc_y*my_p^2
        a128 = singles.tile([P, PAIRS], F32)
        nc.vector.tensor_scalar_mul(out=a128, in0=nmy128, scalar1=scy2)
        m2 = singles.tile([P, PAIRS], F32)
        nc.vector.tensor_mul(out=m2, in0=nmy128, in1=nmy128)
        c128 = singles.tile([P, PAIRS], F32)
        nc.vector.tensor_scalar_mul(out=c128, in0=m2, scalar1=scy128)

        # U(w) = exp(sc_y * w^2), shared by all DVE-path pairs
        w2 = singles.tile([P, W], F32)
        nc.scalar.activation(out=w2, in_=iog,
                             func=mybir.ActivationFunctionType.Square,
                             bias=0.0, scale=1.0)
        ubuf = singles.tile([P, W], F32)
        nc.scalar.activation(out=ubuf, in_=w2,
                             func=mybir.ActivationFunctionType.Exp,
                             bias=0.0, scale=scy128)

        # ---- main loop ---------------------------------------------------
        for p in range(PAIRS):
            st = stage.tile([P, CHUNKS, W], F32)
            if p in ACT_PAIRS:
                # all-ACT path: sq_y then a final Exp per chunk
                sq = ybuf.tile([P, W], F32, tag="sq")
                nc.scalar.activation(
                    out=sq, in_=iog,
                    func=mybir.ActivationFunctionType.Square,
                    bias=nmy128[:, p:p + 1], scale=1.0)
                for c in range(CHUNKS):
                    nc.scalar.activation(
                        out=st[:, c, :], in_=sq,
                        func=mybir.ActivationFunctionType.Exp,
                        bias=lxt[:, c, p:p + 1], scale=scy128)
            else:
                # DVE path: E_p on ACT, U*E_p and scalar mults on DVE
                ep = ybuf.tile([P, W], F32, tag="ep")
                nc.scalar.activation(
                    out=ep, in_=iog,
                    func=mybir.ActivationFunctionType.Exp,
                    bias=c128[:, p:p + 1], scale=a128[:, p:p + 1])
                eyb = ybuf.tile([P, W], F32, tag="eyb")
                nc.vector.tensor_mul(out=eyb, in0=ubuf, in1=ep)
                for c in range(CHUNKS):
                    nc.vector.tensor_scalar_mul(out=st[:, c, :], in0=eyb,
                                                scalar1=ext[:, c, p:p + 1])
            # rows of pair p are h = c*128 + par ; DRAM side iterates
            # (par, c, w) to match the SBUF tile layout.
            nc.sync.dma_start(
                out=out[p * H:(p + 1) * H, :].rearrange(
                    "(c par) w -> par c w", par=P),
                in_=st,
            )
    nc.finalize()
    return nc


LAST_RESULTS = None  # BassKernelResults of the most recent kernel() call


def kernel(x: np.ndarray, labels: np.ndarray,
           log_weight: np.ndarray, **run_kwargs) -> np.ndarray:
    global LAST_RESULTS
    del x  # only its (hardcoded) shape matters
    nc = build_bass()
    labels = np.ascontiguousarray(labels, dtype=np.float32)
    lw = np.ascontiguousarray(log_weight, dtype=np.float32).reshape(1, 1)
    in_maps = [
        {"labels": labels[i * BPC:(i + 1) * BPC], "log_weight": lw}
        for i in range(N_CORES)
    ]
    res = run_bass_kernel_spmd(nc, in_maps, core_ids=list(range(N_CORES)),
                               **run_kwargs)
    LAST_RESULTS = res
    outs = [r["out"].reshape(BPC, NCLS, H, W) for r in res.results]
    return np.concatenate(outs, axis=0)


if __name__ == "__main__":
    rng = np.random.default_rng(0)
    x = rng.standard_normal((B, CH, H, W), dtype=np.float32)
    labels = rng.random((B, 2 * NCLS), dtype=np.float32)
    lw = rng.random((1, 1, 1, 1), dtype=np.float32)
    y = kernel(x=x, labels=labels, log_weight=lw)
    print(y.shape, y.dtype, y.min(), y.max())
